# revision 12
# baseline (speedup 1.0000x reference)
"""Trainium2 Bass kernel for nn_LossSoftDice (soft-dice loss over 32 samples
of 1x512x512 probability/target maps).

Strategy: pure data parallel over the batch. Each of the 8 NeuronCores gets 4
samples (each sample = 262144 f32 elements, viewed as a [128, 2048] tile).
The device computes only per-partition statistics (everything else is
O(128) work done on host during the gather/unshard step):

  inter_p[p]  = sum_f m1[p,f] * m2[p,f]          (DVE tensor_tensor_reduce)
  sum1_p[p]   = sum_f m1[p,f]                     (ACT activation-accumulate)
  sum2_p[p]   = sum_f m2[p,f]                     (ACT activation-accumulate)
  maxp[p]     = max_f m2[p,f]                     (DVE tensor_reduce)
  nsr_p[p]    = #{f : m1[p,f] > 0.5}              (DVE tensor_scalar accum)
  corrl_p[p]  = #{f : (m1[p,f] > 0.5) == (m2[p,f] == maxp[p])}
                                                  (DVE scalar_tensor_tensor)

Host combine (exact, matches the reference's acc branch):
  gmax = max_p maxp[p]
  corr = sum_p ( corrl_p[p]        if maxp[p] == gmax
                 else 2048 - nsr_p[p] )           # all GT==0 in those rows
  score = 2*(inter+1)/(sum1+sum2+1);  score = 1 where corr == 1
  loss = mean(1 - score)
"""

import os
import sys
import types

import numpy as np


def _ensure_concourse():
    try:
        import concourse.bass  # noqa: F401
    except ImportError:
        for p in ("/opt/trn_rl_repo", "/root/.axon_site/_ro/trn_rl_repo"):
            if os.path.isdir(p) and p not in sys.path:
                sys.path.insert(0, p)
        import concourse.bass  # noqa: F401


_ensure_concourse()

import concourse.bass as bass  # noqa: E402
import concourse.bacc as bacc  # noqa: E402
import concourse.tile as tile  # noqa: E402
from concourse import mybir  # noqa: E402
from concourse.bass_utils import run_bass_kernel_spmd  # noqa: E402
from concourse.vector_clock import ScopedClock  # noqa: E402

N_CORES = 8
B = 32                      # total batch
BPC = B // N_CORES          # samples per core
P = 128                     # partitions
F = 2048                    # free dim per partition (P*F = 512*512)

_MAX_WAITS_PER_INST = 1


def _patched_drain_and_barrier(self, tick_clock, wait_clock):
    """Walrus CoreV3Gen rejects CTRL instructions with >2 sem waits; the Tile
    tail drain can carry many. Split them one-per-NoOp before the drain."""
    nc = self.nc
    drain_inst = nc.sync.drain()
    wait_clock.add_sem_waits(
        drain_inst.ins, ScopedClock({None: tick_clock.global_clock})
    )
    si = drain_inst.ins.sync_info
    if si is not None and si.on_wait and len(si.on_wait) > _MAX_WAITS_PER_INST:
        waits = list(si.on_wait)
        si.on_wait = waits[:_MAX_WAITS_PER_INST]
        insts = nc.cur_bb.bb.instructions
        assert insts[-1] is drain_inst.ins
        nops = []
        for w in waits[_MAX_WAITS_PER_INST:]:
            nop_inst = nc.sync.nop(nofuse=True, hint="drain_wait_split")
            if nop_inst.ins.sync_info is None:
                nop_inst.ins.sync_info = mybir.SyncInfo(on_wait=[], on_update=[])
            nop_inst.ins.sync_info.on_wait.append(w)
            nops.append(insts.pop())
        d = insts.pop()
        insts.extend(nops)
        insts.append(d)

    nc.all_engine_barrier()
    assert self.sems is not None
    popped = nc._tile_sem_poison_stack.pop()
    assert popped is self._sem_poison
    nc.clear_and_free_semaphores(list(self.sems.allocated().values()))
    nc.all_engine_barrier()


def _slim_drain_and_barrier(self, tick_clock, wait_clock):
    # Same as TileContext._drain_and_barrier but without the second
    # all-engine barrier: NRT itself waits for every engine to halt before
    # the NEFF can be re-executed, so the sem clear does not need another
    # intra-NEFF barrier after it. (Bacc.compile legalizes multi-waits.)
    nc = self.nc
    drain_inst = nc.sync.drain()
    wait_clock.add_sem_waits(
        drain_inst.ins, ScopedClock({None: tick_clock.global_clock})
    )
    nc.all_engine_barrier()
    assert self.sems is not None
    popped = nc._tile_sem_poison_stack.pop()
    assert popped is self._sem_poison
    nc.clear_and_free_semaphores(list(self.sems.allocated().values()))


tile.TileContext._drain_and_barrier = _slim_drain_and_barrier


def _install_ntff_hook_module():
    """bass_utils imports antenv.axon_hooks when trace=True under axon; this
    container's antenv lacks that module. Recreate it from the boot helper."""
    if "antenv.axon_hooks" in sys.modules:
        return
    try:
        import trn_agent_boot.trn_boot as tb

        hook = tb._ntff_profile_via_ctypes("/opt/axon/libaxon_pjrt.so")
    except Exception:
        hook = None
    m = types.ModuleType("antenv.axon_hooks")
    m.get_axon_ntff_profile_hook = lambda: hook
    m.set_axon_ntff_profile_hook = lambda h: None
    sys.modules["antenv.axon_hooks"] = m


_STAT_NAMES = ("inter", "den", "maxp", "nsr")


def _build_nc():
    nc = bacc.Bacc("TRN2", debug=False)
    f32 = mybir.dt.float32
    probs = nc.dram_tensor("probs", [BPC, P, F], f32, kind="ExternalInput").ap()
    targets = nc.dram_tensor("targets", [BPC, P, F], f32, kind="ExternalInput").ap()
    stats_out = nc.dram_tensor(
        "stats", [P, 4 * BPC], f32, kind="ExternalOutput"
    ).ap()

    A = mybir.AluOpType
    with tile.TileContext(nc) as tc:
        with (
            tc.tile_pool(name="m1", bufs=BPC) as m1_pool,
            tc.tile_pool(name="m2", bufs=BPC) as m2_pool,
            tc.tile_pool(name="scr", bufs=1) as scr_pool,
            tc.tile_pool(name="sr", bufs=2) as sr_pool,
            tc.tile_pool(name="stats", bufs=1) as stats_pool,
        ):
            mds = []
            HP = P // 2
            for s in range(BPC):
                md = m1_pool.tile([P, 2 * F], f32, tag="md", name=f"md{s}")
                # Each tensor loads as two partition-half DMAs on the two
                # HWDGE rings: p0-63 hit the even SDMA engines, p64-127 the
                # odd ones, so both rings stream one tensor concurrently and
                # ring-FIFO order completes sample s fully before sample s+1.
                nc.sync.dma_start(md[0:HP, 0:F], targets[s][0:HP, :])
                nc.scalar.dma_start(md[HP:P, 0:F], targets[s][HP:P, :])
                nc.sync.dma_start(md[0:HP, F : 2 * F], probs[s][0:HP, :])
                nc.scalar.dma_start(md[HP:P, F : 2 * F], probs[s][HP:P, :])
                mds.append(md)

            dve_scr = scr_pool.tile([P, F], f32, tag="dve_scr")
            act_scr = scr_pool.tile([P, 2 * F], f32, tag="act_scr")
            st_tile = stats_pool.tile(
                [P, 4 * BPC], f32, tag="st", name="st_all"
            )
            st = {
                name: st_tile[:, j * BPC : (j + 1) * BPC]
                for j, name in enumerate(_STAT_NAMES)
            }

            for s in range(BPC):
                md = mds[s]
                m2 = md[:, 0:F]
                m1 = md[:, F : 2 * F]
                c = slice(s, s + 1)
                # per-partition max of targets (needs only m2 -> starts first)
                nc.vector.tensor_reduce(
                    st["maxp"][:, c], m2, mybir.AxisListType.X, A.max
                )
                # denominator: per-partition sum of (m2|m1) in one ACT pass
                nc.scalar.activation(
                    act_scr[:], md[:], mybir.ActivationFunctionType.Copy,
                    accum_out=st["den"][:, c],
                )
                if s == BPC - 1:
                    # balance: last sample counts SR on DVE (accum variant)
                    sr = sr_pool.tile([P, F], f32, tag="sr")
                    nc.vector.tensor_scalar(
                        sr[:], m1, 0.5, None, A.is_gt, A.add,
                        accum_out=st["nsr"][:, c],
                    )
                else:
                    # SR = m1 > 0.5 (plain tensor_scalar -> 2x DVE mode),
                    # counted on the scalar engine
                    sr = sr_pool.tile([P, F], f32, tag="sr")
                    nc.vector.tensor_scalar(sr[:], m1, 0.5, None, A.is_gt)
                    nc.scalar.activation(
                        act_scr[:, 0:F], sr[:], mybir.ActivationFunctionType.Copy,
                        accum_out=st["nsr"][:, c],
                    )
                # intersection per partition (+ throwaway product tile)
                nc.vector.scalar_tensor_tensor(
                    out=dve_scr[:],
                    in0=m1,
                    scalar=1.0,
                    in1=m2,
                    op0=A.mult,
                    op1=A.mult,
                    accum_out=st["inter"][:, c],
                )

            nc.sync.dma_start(stats_out, st_tile[:])

    nc.compile()
    return nc


def _shard_inputs(probs, targets):
    probs = np.ascontiguousarray(np.asarray(probs, dtype=np.float32)).reshape(B, P, F)
    targets = np.ascontiguousarray(np.asarray(targets, dtype=np.float32)).reshape(
        B, P, F
    )
    in_maps = []
    for i in range(N_CORES):
        sl = slice(i * BPC, (i + 1) * BPC)
        in_maps.append(
            {
                "probs": np.ascontiguousarray(probs[sl]),
                "targets": np.ascontiguousarray(targets[sl]),
            }
        )
    return in_maps


def _combine(results, probs, targets):
    """Exact host-side combine of per-partition stats -> scalar loss.

    corr_b = N - nSR - K + 2A with K (#elements == global max) and
    A (#those with m1 > 0.5) recovered by scanning only the partitions
    that attain the global max (O(2048) per sample, exact)."""
    inter = np.empty(B)
    den = np.empty(B)
    corr = np.empty(B)
    N = float(P * F)
    for i in range(N_CORES):
        r = results[i]["stats"]
        col = {name: r[:, j * BPC : (j + 1) * BPC] for j, name in enumerate(_STAT_NAMES)}
        for s in range(BPC):
            b = i * BPC + s
            inter[b] = col["inter"][:, s].astype(np.float64).sum()
            den[b] = col["den"][:, s].astype(np.float64).sum()
            nsr = col["nsr"][:, s].astype(np.float64).sum()
            maxp = col["maxp"][:, s]
            gmax = maxp.max()
            K = A = 0
            for p in np.nonzero(maxp == gmax)[0]:
                hit = targets[b, p, :] == gmax
                K += int(hit.sum())
                A += int((hit & (probs[b, p, :] > 0.5)).sum())
            corr[b] = N - nsr - K + 2 * A
    score = 2.0 * (inter + 1.0) / (den + 1.0)
    score = np.where(corr == 1.0, 1.0, score)
    return np.array(np.mean(1.0 - score), dtype=np.float32)


def _run(probs, targets, trace=False, tmpdir=None):
    if trace:
        _install_ntff_hook_module()
    nc = _build_nc()
    in_maps = _shard_inputs(probs, targets)
    res = run_bass_kernel_spmd(
        nc, in_maps, list(range(N_CORES)), trace=trace, tmpdir=tmpdir
    )
    pr = np.asarray(probs, dtype=np.float32).reshape(B, P, F)
    tg = np.asarray(targets, dtype=np.float32).reshape(B, P, F)
    out = _combine(res.results, pr, tg)
    return out, res


def kernel(probs, targets):
    out, _ = _run(probs, targets)
    return out


# revision 13
# speedup vs baseline: 1.3158x; 1.3158x over previous
"""Trainium2 Bass kernel for nn_LossSoftDice (soft-dice loss over 32 samples
of 1x512x512 probability/target maps).

Strategy: pure data parallel over the batch. Each of the 8 NeuronCores gets 4
samples (each sample = 262144 f32 elements, viewed as a [128, 2048] tile).
The device computes only per-partition statistics (everything else is
O(128) work done on host during the gather/unshard step):

  inter_p[p]  = sum_f m1[p,f] * m2[p,f]          (DVE tensor_tensor_reduce)
  sum1_p[p]   = sum_f m1[p,f]                     (ACT activation-accumulate)
  sum2_p[p]   = sum_f m2[p,f]                     (ACT activation-accumulate)
  maxp[p]     = max_f m2[p,f]                     (DVE tensor_reduce)
  nsr_p[p]    = #{f : m1[p,f] > 0.5}              (DVE tensor_scalar accum)
  corrl_p[p]  = #{f : (m1[p,f] > 0.5) == (m2[p,f] == maxp[p])}
                                                  (DVE scalar_tensor_tensor)

Host combine (exact, matches the reference's acc branch):
  gmax = max_p maxp[p]
  corr = sum_p ( corrl_p[p]        if maxp[p] == gmax
                 else 2048 - nsr_p[p] )           # all GT==0 in those rows
  score = 2*(inter+1)/(sum1+sum2+1);  score = 1 where corr == 1
  loss = mean(1 - score)
"""

import os
import sys
import types

import numpy as np


def _ensure_concourse():
    try:
        import concourse.bass  # noqa: F401
    except ImportError:
        for p in ("/opt/trn_rl_repo", "/root/.axon_site/_ro/trn_rl_repo"):
            if os.path.isdir(p) and p not in sys.path:
                sys.path.insert(0, p)
        import concourse.bass  # noqa: F401


_ensure_concourse()

import concourse.bass as bass  # noqa: E402
import concourse.bacc as bacc  # noqa: E402
import concourse.tile as tile  # noqa: E402
from concourse import mybir  # noqa: E402
from concourse.bass_utils import run_bass_kernel_spmd  # noqa: E402
from concourse.vector_clock import ScopedClock  # noqa: E402

N_CORES = 8
B = 32                      # total batch
BPC = B // N_CORES          # samples per core
P = 128                     # partitions
F = 2048                    # free dim per partition (P*F = 512*512)

_MAX_WAITS_PER_INST = 1


def _patched_drain_and_barrier(self, tick_clock, wait_clock):
    """Walrus CoreV3Gen rejects CTRL instructions with >2 sem waits; the Tile
    tail drain can carry many. Split them one-per-NoOp before the drain."""
    nc = self.nc
    drain_inst = nc.sync.drain()
    wait_clock.add_sem_waits(
        drain_inst.ins, ScopedClock({None: tick_clock.global_clock})
    )
    si = drain_inst.ins.sync_info
    if si is not None and si.on_wait and len(si.on_wait) > _MAX_WAITS_PER_INST:
        waits = list(si.on_wait)
        si.on_wait = waits[:_MAX_WAITS_PER_INST]
        insts = nc.cur_bb.bb.instructions
        assert insts[-1] is drain_inst.ins
        nops = []
        for w in waits[_MAX_WAITS_PER_INST:]:
            nop_inst = nc.sync.nop(nofuse=True, hint="drain_wait_split")
            if nop_inst.ins.sync_info is None:
                nop_inst.ins.sync_info = mybir.SyncInfo(on_wait=[], on_update=[])
            nop_inst.ins.sync_info.on_wait.append(w)
            nops.append(insts.pop())
        d = insts.pop()
        insts.extend(nops)
        insts.append(d)

    nc.all_engine_barrier()
    assert self.sems is not None
    popped = nc._tile_sem_poison_stack.pop()
    assert popped is self._sem_poison
    nc.clear_and_free_semaphores(list(self.sems.allocated().values()))
    nc.all_engine_barrier()


def _slim_drain_and_barrier(self, tick_clock, wait_clock):
    # Same as TileContext._drain_and_barrier but without the second
    # all-engine barrier: NRT itself waits for every engine to halt before
    # the NEFF can be re-executed, so the sem clear does not need another
    # intra-NEFF barrier after it. (Bacc.compile legalizes multi-waits.)
    nc = self.nc
    drain_inst = nc.sync.drain()
    wait_clock.add_sem_waits(
        drain_inst.ins, ScopedClock({None: tick_clock.global_clock})
    )
    nc.all_engine_barrier()
    assert self.sems is not None
    popped = nc._tile_sem_poison_stack.pop()
    assert popped is self._sem_poison
    nc.clear_and_free_semaphores(list(self.sems.allocated().values()))


tile.TileContext._drain_and_barrier = _slim_drain_and_barrier


def _install_ntff_hook_module():
    """bass_utils imports antenv.axon_hooks when trace=True under axon; this
    container's antenv lacks that module. Recreate it from the boot helper."""
    if "antenv.axon_hooks" in sys.modules:
        return
    try:
        import trn_agent_boot.trn_boot as tb

        hook = tb._ntff_profile_via_ctypes("/opt/axon/libaxon_pjrt.so")
    except Exception:
        hook = None
    m = types.ModuleType("antenv.axon_hooks")
    m.get_axon_ntff_profile_hook = lambda: hook
    m.set_axon_ntff_profile_hook = lambda h: None
    sys.modules["antenv.axon_hooks"] = m


_STAT_NAMES = ("inter", "den", "maxp", "nsr")


def _build_nc():
    nc = bacc.Bacc("TRN2", debug=False)
    f32 = mybir.dt.float32
    probs = nc.dram_tensor("probs", [BPC, P, F], f32, kind="ExternalInput").ap()
    targets = nc.dram_tensor("targets", [BPC, P, F], f32, kind="ExternalInput").ap()
    stats_out = nc.dram_tensor(
        "stats", [P, 4 * BPC], f32, kind="ExternalOutput"
    ).ap()

    A = mybir.AluOpType
    with tile.TileContext(nc) as tc:
        with (
            tc.tile_pool(name="m1", bufs=BPC) as m1_pool,
            tc.tile_pool(name="m2", bufs=BPC) as m2_pool,
            tc.tile_pool(name="scr", bufs=1) as scr_pool,
            tc.tile_pool(name="sr", bufs=2) as sr_pool,
            tc.tile_pool(name="stats", bufs=1) as stats_pool,
        ):
            mds = []
            for s in range(BPC):
                md = m1_pool.tile([P, 2 * F], f32, tag="md", name=f"md{s}")
                # m2 in the low half (sync ring), m1 in the high half
                # (scalar ring) - two HWDGE rings dispatch in parallel.
                nc.sync.dma_start(md[:, 0:F], targets[s])
                nc.scalar.dma_start(md[:, F : 2 * F], probs[s])
                mds.append(md)

            dve_scr = scr_pool.tile([P, F], f32, tag="dve_scr")
            act_scr = scr_pool.tile([P, 2 * F], f32, tag="act_scr")
            st_tile = stats_pool.tile(
                [P, 4 * BPC], f32, tag="st", name="st_all"
            )
            st = {
                name: st_tile[:, j * BPC : (j + 1) * BPC]
                for j, name in enumerate(_STAT_NAMES)
            }

            for s in range(BPC):
                md = mds[s]
                m2 = md[:, 0:F]
                m1 = md[:, F : 2 * F]
                c = slice(s, s + 1)
                # per-partition max of targets (needs only m2 -> starts first)
                nc.vector.tensor_reduce(
                    st["maxp"][:, c], m2, mybir.AxisListType.X, A.max
                )
                # denominator: per-partition sum of (m2|m1) in one ACT pass
                nc.scalar.activation(
                    act_scr[:], md[:], mybir.ActivationFunctionType.Copy,
                    accum_out=st["den"][:, c],
                )
                if s == BPC - 1:
                    # balance: last sample counts SR on DVE (accum variant)
                    sr = sr_pool.tile([P, F], f32, tag="sr")
                    nc.vector.tensor_scalar(
                        sr[:], m1, 0.5, None, A.is_gt, A.add,
                        accum_out=st["nsr"][:, c],
                    )
                else:
                    # SR = m1 > 0.5 (plain tensor_scalar -> 2x DVE mode),
                    # counted on the scalar engine
                    sr = sr_pool.tile([P, F], f32, tag="sr")
                    nc.vector.tensor_scalar(sr[:], m1, 0.5, None, A.is_gt)
                    nc.scalar.activation(
                        act_scr[:, 0:F], sr[:], mybir.ActivationFunctionType.Copy,
                        accum_out=st["nsr"][:, c],
                    )
                # intersection per partition (+ throwaway product tile)
                nc.vector.scalar_tensor_tensor(
                    out=dve_scr[:],
                    in0=m1,
                    scalar=1.0,
                    in1=m2,
                    op0=A.mult,
                    op1=A.mult,
                    accum_out=st["inter"][:, c],
                )

            nc.sync.dma_start(stats_out, st_tile[:])

    nc.compile()
    return nc


def _shard_inputs(probs, targets):
    probs = np.ascontiguousarray(np.asarray(probs, dtype=np.float32)).reshape(B, P, F)
    targets = np.ascontiguousarray(np.asarray(targets, dtype=np.float32)).reshape(
        B, P, F
    )
    in_maps = []
    for i in range(N_CORES):
        sl = slice(i * BPC, (i + 1) * BPC)
        in_maps.append(
            {
                "probs": np.ascontiguousarray(probs[sl]),
                "targets": np.ascontiguousarray(targets[sl]),
            }
        )
    return in_maps


def _combine(results, probs, targets):
    """Exact host-side combine of per-partition stats -> scalar loss.

    corr_b = N - nSR - K + 2A with K (#elements == global max) and
    A (#those with m1 > 0.5) recovered by scanning only the partitions
    that attain the global max (O(2048) per sample, exact)."""
    inter = np.empty(B)
    den = np.empty(B)
    corr = np.empty(B)
    N = float(P * F)
    for i in range(N_CORES):
        r = results[i]["stats"]
        col = {name: r[:, j * BPC : (j + 1) * BPC] for j, name in enumerate(_STAT_NAMES)}
        for s in range(BPC):
            b = i * BPC + s
            inter[b] = col["inter"][:, s].astype(np.float64).sum()
            den[b] = col["den"][:, s].astype(np.float64).sum()
            nsr = col["nsr"][:, s].astype(np.float64).sum()
            maxp = col["maxp"][:, s]
            gmax = maxp.max()
            K = A = 0
            for p in np.nonzero(maxp == gmax)[0]:
                hit = targets[b, p, :] == gmax
                K += int(hit.sum())
                A += int((hit & (probs[b, p, :] > 0.5)).sum())
            corr[b] = N - nsr - K + 2 * A
    score = 2.0 * (inter + 1.0) / (den + 1.0)
    score = np.where(corr == 1.0, 1.0, score)
    return np.array(np.mean(1.0 - score), dtype=np.float32)


def _run(probs, targets, trace=False, tmpdir=None):
    _install_ntff_hook_module()
    nc = _build_nc()
    in_maps = _shard_inputs(probs, targets)
    res = run_bass_kernel_spmd(
        nc, in_maps, list(range(N_CORES)), trace=trace, tmpdir=tmpdir
    )
    pr = np.asarray(probs, dtype=np.float32).reshape(B, P, F)
    tg = np.asarray(targets, dtype=np.float32).reshape(B, P, F)
    out = _combine(res.results, pr, tg)
    return out, res


def kernel(probs, targets):
    out, _ = _run(probs, targets)
    return out
